# revision 2
# baseline (speedup 1.0000x reference)
"""AdaptiveHadamardTransform on 8 TRN2 NeuronCores.

y = scale * FHT_4096(x) + shift, x: (4, 4096, 4096) f32.

Algorithm: H_4096 = H_32 (x) H_128 (Sylvester Kronecker factorization).
Each 4096-row, viewed as X[i, k] (i in [0,32), k in [0,128)), transforms as
    y[i', k'] = sum_{i,k} H32[i, i'] * H128[k, k'] * X[i, k]
Two chained TensorEngine matmuls with the DATA as the stationary (lhsT)
operand do both contractions with no transposes:
  stage 1:  out1[k, (t',i')]  = sum_{(t,i)} A[(t,i), k] * blockdiag4(H32)
  stage 2:  out2[(t',i'), k'] = sum_k      out1[k, ...] * H128
where t in [0,4) packs 4 rows per 128-partition tile so the contraction
uses all 128 partitions.

The datapath runs in bf16 (tolerance is 2e-2; bf16 keeps rel err ~4e-3):
 - matmuls run at 1 cycle/row instead of fp32's 4
 - HBM traffic halves (16 MiB in + 16 MiB out per core)
The host pre-packs x into the per-core tile layout [128(t,i), 512 r, 128 k]
in bf16 so every DMA transfer is contiguous 1 KiB per partition, and
unpacks the bf16 output back to fp32.

Sharding: data-parallel over the 16384 rows -> 2048 rows per core;
scale/shift folded into per-tile constants, replicated to all cores.
"""

import sys

sys.path.insert(0, "/opt/trn_rl_repo")

import numpy as np
import ml_dtypes

BF16 = ml_dtypes.bfloat16

SIZE = 4096
N_CORES = 8
ROWS = 16384  # 4 * 4096
ROWS_PER_CORE = ROWS // N_CORES  # 2048
R_VALS = ROWS_PER_CORE // 4  # 512 "r" values (4 rows each)

_CACHE = {}


def _sylvester(m: int) -> np.ndarray:
    H = np.array([[1.0]], dtype=np.float32)
    for _ in range(m):
        H = np.block([[H, H], [H, -H]]).astype(np.float32)
    return H


def _build_nc():
    import concourse.mybir as mybir
    from concourse import bacc, tile

    f32 = mybir.dt.float32
    bf16 = mybir.dt.bfloat16
    nc = bacc.Bacc("TRN2", target_bir_lowering=False, debug=False, num_devices=N_CORES)

    # Pre-packed input: [p=(t,i), r, k] with p = t*32 + i, element = row
    # (4r+t), column i*128+k of the core's 2048x4096 slab.
    x = nc.dram_tensor("x", [128, R_VALS, 128], bf16, kind="ExternalInput").ap()
    hbd4 = nc.dram_tensor("hbd4", [128, 128], bf16, kind="ExternalInput").ap()
    h128 = nc.dram_tensor("h128", [128, 128], bf16, kind="ExternalInput").ap()
    stl = nc.dram_tensor("stile", [128, 512], f32, kind="ExternalInput").ap()
    btl = nc.dram_tensor("btile", [128, 512], bf16, kind="ExternalInput").ap()
    out = nc.dram_tensor("out", [128, R_VALS, 128], bf16, kind="ExternalOutput").ap()

    with tile.TileContext(nc) as tc:
        with (
            tc.tile_pool(name="consts", bufs=1) as cpool,
            tc.tile_pool(name="a", bufs=12) as apool,
            tc.tile_pool(name="s1", bufs=4) as spool,
            tc.tile_pool(name="ot", bufs=4) as opool,
            tc.tile_pool(name="ps1", bufs=3, space="PSUM") as ppool1,
            tc.tile_pool(name="ps2", bufs=3, space="PSUM") as ppool2,
        ):
            hbd_t = cpool.tile([128, 128], bf16)
            nc.scalar.dma_start(hbd_t[:], hbd4[:])
            h128_t = cpool.tile([128, 128], bf16)
            nc.scalar.dma_start(h128_t[:], h128[:])
            st_t = cpool.tile([128, 512], f32)
            nc.scalar.dma_start(st_t[:], stl[:])
            bt_t = cpool.tile([128, 512], bf16)
            nc.scalar.dma_start(bt_t[:], btl[:])

            def stage2(s1, g_abs):
                """Emit stage-2 matmuls + scale/shift + out-DMA for group g_abs."""
                p2 = ppool2.tile([128, 512], f32)
                for u in range(4):
                    nc.tensor.matmul(
                        p2[:, u * 128 : (u + 1) * 128],
                        s1[:, u * 128 : (u + 1) * 128],
                        h128_t[:],
                        start=True,
                        stop=True,
                    )
                ot = opool.tile([128, 4, 128], bf16)
                otf = ot[:].rearrange("p r k -> p (r k)")
                nc.vector.tensor_mul(otf, p2[:], st_t[:])
                nc.vector.tensor_add(otf, otf, bt_t[:])
                r0 = g_abs * 4
                nc.scalar.dma_start(out[:, r0 : r0 + 4, :], ot[:])

            # Software-pipelined: stage 2 of group g-1 is emitted after
            # stage 1 of group g, so the in-order PE queue never waits on
            # the interstage ACT copy.
            pend = None  # (s1_tile, g_abs)
            for ga in range(R_VALS // 4):  # 128 groups of 4 r (16 rows) each
                a_t = apool.tile([128, 4, 128], bf16)
                if ga < 2:
                    # fine-grained first loads: descriptor generation for a
                    # full-group load delays the first matmul
                    for u in range(4):
                        nc.sync.dma_start(
                            a_t[:, u : u + 1, :], x[:, ga * 4 + u : ga * 4 + u + 1, :]
                        )
                else:
                    nc.sync.dma_start(a_t[:], x[:, ga * 4 : (ga + 1) * 4, :])
                p1 = ppool1.tile([128, 512], f32)
                for u in range(4):
                    nc.tensor.matmul(
                        p1[:, u * 128 : (u + 1) * 128],
                        a_t[:, u, :],
                        hbd_t[:],
                        start=True,
                        stop=True,
                    )
                s1 = spool.tile([128, 512], bf16)
                nc.scalar.copy(s1[:], p1[:])
                if pend is not None:
                    stage2(*pend)
                pend = (s1, ga)
            stage2(*pend)
    nc.compile()
    return nc


def _get_nc():
    if "nc" not in _CACHE:
        _CACHE["nc"] = _build_nc()
    return _CACHE["nc"]


def _make_const_tiles(scale: np.ndarray, shift: np.ndarray):
    H32 = _sylvester(5)
    H128 = _sylvester(7)
    hbd4 = np.zeros((128, 128), dtype=np.float32)
    for t in range(4):
        hbd4[t * 32 : (t + 1) * 32, t * 32 : (t + 1) * 32] = H32
    pp = np.arange(128) % 32  # i' index per partition
    ff = np.arange(512) % 128  # k' index per free column
    s2d = (scale.astype(np.float32) / 64.0).reshape(32, 128)
    b2d = shift.astype(np.float32).reshape(32, 128)
    s_tile = np.ascontiguousarray(s2d[pp][:, ff])
    b_tile = np.ascontiguousarray(b2d[pp][:, ff]).astype(BF16)
    return hbd4.astype(BF16), H128.astype(BF16), s_tile, b_tile


def _pack_core(xc16: np.ndarray) -> np.ndarray:
    """[2048, 4096] bf16 -> [128 (t,i), 512 r, 128 k] bf16 (contiguous)."""
    v = xc16.reshape(R_VALS, 4, 32, 128)  # r, t, i, k
    return np.ascontiguousarray(v.transpose(1, 2, 0, 3)).reshape(128, R_VALS, 128)


def _unpack_core(oc: np.ndarray) -> np.ndarray:
    """[128 (t,i'), 512 r, 128 k'] bf16 -> [2048, 4096] f32."""
    v = oc.reshape(4, 32, R_VALS, 128).transpose(2, 0, 1, 3)  # r, t, i', k'
    return v.reshape(ROWS_PER_CORE, SIZE).astype(np.float32)


def kernel(x: np.ndarray, scale: np.ndarray, shift: np.ndarray) -> np.ndarray:
    from concourse.bass_utils import run_bass_kernel_spmd

    x = np.asarray(x)
    scale = np.asarray(scale)
    shift = np.asarray(shift)
    nc = _get_nc()
    xf = x.reshape(ROWS, SIZE).astype(BF16)
    hbd4, H128, s_tile, b_tile = _make_const_tiles(scale, shift)

    in_maps = []
    for c in range(N_CORES):
        in_maps.append(
            {
                "x": _pack_core(xf[c * ROWS_PER_CORE : (c + 1) * ROWS_PER_CORE]),
                "hbd4": hbd4,
                "h128": H128,
                "stile": s_tile,
                "btile": b_tile,
            }
        )
    res = run_bass_kernel_spmd(nc, in_maps, core_ids=list(range(N_CORES)))
    out = np.concatenate(
        [_unpack_core(res.results[c]["out"]) for c in range(N_CORES)], axis=0
    )
    return out.reshape(x.shape)
